# revision 51
# baseline (speedup 1.0000x reference)
"""Trainium2 Bass kernel for nn_Guesser_2559800508528 (sparse_attention), v3.

Math (restructured from the reference):
  ghT[d,b]  = sum_k G[b,k,d] / K          (DVE grouped reduce on fp8 G^T)
  a1n[b,h]  = ghT @ W1q + b1              (PE bf16, [b,h] layout, b1 via ones-row)
  h[h,b*t]  = relu(X^T(fp8) @ W1x(fp8) [DoubleRow] + a1n-inject)  -> h8 (fp8)
  e[1,rows] = w2 . h8                     (PE fp8 DoubleRow; 4 rts batched into
                                           one PSUM bank at partitions {0,32,64,96})
  u = exp(e); alpha = u / sum_t u         (batched [128,512] strips)
  xhT[d,b]  = sum_t alpha[t] * X^T[d,t]   (gpsimd mult + DVE pairwise tree)
  a2[b,j]   = xhT @ M1x' + c1'            (PE bf16; m2-sign-permuted, S-scaled)
  h2[row,j] = relu(G^T(fp8) @ M1g'(fp8) [DoubleRow] + a2-inject)
  logits    = (sum_j signed-relu accums)/S + c2   (ACT/DVE accum_out)

M1g'/M1x'/c1' are scaled by S=1024 so the m2-folded fp8 weights stay in
e4m3 range; the final strip op multiplies by 1/S.

Sharding: pure data parallel over batch (512 batches per core x 8 cores).
"""

import numpy as np
import ml_dtypes

import concourse.bass as bass
import concourse.mybir as mybir
import concourse.tile as tile
from concourse import bacc
from concourse.bass_utils import run_bass_kernel_spmd

dt = mybir.dt
AF = mybir.ActivationFunctionType
ALU = mybir.AluOpType
F8 = ml_dtypes.float8_e4m3
BF = ml_dtypes.bfloat16

NCORES = 8
B_CORE = 512
T = 32
K = 16
D = 512
H = 1024
H2 = 2048
NX = B_CORE * T          # 16384 X rows per core
NG = B_CORE * K          # 8192 G rows per core
RT_X = NX // 512         # 32 row-tiles of 512 rows (16 batches)
RT_G = NG // 512         # 16 row-tiles of 512 rows (32 batches)
S_F = 1024.0             # fp8-range scale on M1g'/M1x'/c1'

# ---- engine-split knobs ----
INJ_SLICES = frozenset(range(8))   # h-slices whose a1 is injected on PE
BATCH_SOFTMAX = False              # PSUM out partition offsets are ISA-illegal
C_EVAC_DVE = frozenset({6, 7})     # (rt+s)%8 values whose h-evac runs on DVE
F_POS_DVE = frozenset()            # (rt*4+c)%4 values whose pos-range runs on DVE
F_NEG_ACT = False                  # neg ranges on DVE signed-min (bit-equiv, fewer ops)
DUM_SBUF = False                   # accum-op dst: SBUF dum tiles vs in-place PSUM
DEBUG_XHT = False                  # extra debug outputs (xhT/ghT/a2n)


def build_nc(P, reps=1):
    """P = sign boundary; reps>1 repeats the compute body (timing builds)."""
    nc = bacc.Bacc("TRN2", target_bir_lowering=False, debug=False)

    d = {}
    d["xt8"] = nc.declare_dram_parameter("XT8", [256, 2 * NX], dt.float8e4, isOutput=False).ap()
    d["xtb"] = nc.declare_dram_parameter("XTB", [128, 4 * NX], dt.bfloat16, isOutput=False).ap()
    d["gtb"] = nc.declare_dram_parameter("GTB", [128, 4 * NG], dt.bfloat16, isOutput=False).ap()
    d["w1x8"] = nc.declare_dram_parameter("W1X8", [256, 2048], dt.float8e4, isOutput=False).ap()
    d["w1qb"] = nc.declare_dram_parameter("W1QB", [512, 1024], dt.bfloat16, isOutput=False).ap()
    d["w28"] = nc.declare_dram_parameter("W28", [128, 256], dt.float8e4, isOutput=False).ap()
    d["m1xb"] = nc.declare_dram_parameter("M1XB", [512, 2048], dt.bfloat16, isOutput=False).ap()
    d["m1gb"] = nc.declare_dram_parameter("M1GB", [512, 2048], dt.bfloat16, isOutput=False).ap()
    d["c1r"] = nc.declare_dram_parameter("C1R", [1, 2048], dt.bfloat16, isOutput=False).ap()
    d["b1"] = nc.declare_dram_parameter("B1", [1024, 1], dt.float32, isOutput=False).ap()
    d["b1r"] = nc.declare_dram_parameter("B1R", [1, 1024], dt.bfloat16, isOutput=False).ap()
    d["ind32"] = nc.declare_dram_parameter("IND32", [128, 2048], dt.bfloat16, isOutput=False).ap()
    d["bindf"] = nc.declare_dram_parameter("BINDF", [128, 512], dt.bfloat16, isOutput=False).ap()
    d["c2"] = nc.declare_dram_parameter("C2", [1, 1], dt.float32, isOutput=False).ap()
    d["out"] = nc.declare_dram_parameter("out", [128, 64], dt.float32, isOutput=True).ap()
    if DEBUG_XHT:
        d["xht_out"] = nc.declare_dram_parameter(
            "xht_out", [128, 2048], dt.bfloat16, isOutput=True).ap()
        d["ght_out"] = nc.declare_dram_parameter(
            "ght_out", [128, 2048], dt.bfloat16, isOutput=True).ap()
        d["a2n_out"] = nc.declare_dram_parameter(
            "a2n_out", [64, 2048], dt.bfloat16, isOutput=True).ap()
        d["u_out"] = nc.declare_dram_parameter(
            "u_out", [128, 512], dt.bfloat16, isOutput=True).ap()
        d["xu_out"] = nc.declare_dram_parameter(
            "xu_out", [128, 2048], dt.bfloat16, isOutput=True).ap()
        d["h8_out"] = nc.declare_dram_parameter(
            "h8_out", [128, 1024], dt.float8e4, isOutput=True).ap()
        d["a1n_out"] = nc.declare_dram_parameter(
            "a1n_out", [128, 1024], dt.bfloat16, isOutput=True).ap()

    with tile.TileContext(nc) as tc:
        _body(nc, tc, P, d, reps)
    nc.compile()
    return nc


def _f_ranges(P):
    """Signed accum ranges (blk, a, b, col, sign) given sign boundary P.

    Columns of M1g'/M1x'/c1' are scaled by SIGNED m2 (and S): positive-m2
    columns contribute relu(pre') (sign +1); negative-m2 columns contribute
    min(pre', 0) (sign +1, value already negative) or relu(-pre') (sign -1)
    when accumulated on ACT.
    """
    rngs = []
    col = 0
    for blk in range(2):
        lo, hi = 1024 * blk, 1024 * blk + 1024
        pa, pb_ = lo, min(P, hi)
        if pb_ > pa:
            rngs.append((blk, pa - lo, pb_ - lo, col, "pos"))
            col += 1
        na, nb = max(P, lo), hi
        if nb > na:
            rngs.append((blk, na - lo, nb - lo, col, "neg"))
            col += 1
    return rngs, col


def _body(nc, tc, P, dr, reps=1):
    from contextlib import ExitStack
    ctx = ExitStack()
    f_ranges, n_acc = _f_ranges(P)
    inj_all = INJ_SLICES == frozenset(range(8))
    inj_none = not INJ_SLICES
    with ctx:
        persist = ctx.enter_context(tc.tile_pool(name="persist", bufs=1))
        # ---- persistent weights (direct DMA, pre-cast on host) ----
        w1x8 = [persist.tile([128, 2048], dt.float8e4, tag=f"w1x8_{p}", name=f"w1x8_{p}")
                for p in range(2)]
        for p in range(2):
            nc.sync.dma_start(w1x8[p][:], dr["w1x8"][128 * p:128 * (p + 1), :])
        w1qb = [persist.tile([128, 1024], dt.bfloat16, tag=f"w1qb_{d_}", name=f"w1qb_{d_}")
                for d_ in range(4)]
        for d_ in range(4):
            nc.sync.dma_start(w1qb[d_][:], dr["w1qb"][128 * d_:128 * (d_ + 1), :])
        m1xb = [persist.tile([128, 2048], dt.bfloat16, tag=f"m1xb_{d_}", name=f"m1xb_{d_}")
                for d_ in range(4)]
        m1gb = [persist.tile([128, 2048], dt.bfloat16, tag=f"m1gb_{d_}", name=f"m1gb_{d_}")
                for d_ in range(4)]
        w28 = persist.tile([128, 256], dt.float8e4, tag="w28", name="w28")
        nc.sync.dma_start(w28[:], dr["w28"][:, :])
        bindfh = [persist.tile([64, 512], dt.bfloat16, tag=f"bindf_{j}",
                               name=f"bindf_{j}") for j in range(2)]
        for j in range(2):
            nc.sync.dma_start(bindfh[j][:], dr["bindf"][64 * j:64 * (j + 1), :])
        c1r = persist.tile([1, 2048], dt.bfloat16, tag="c1r", name="c1r")
        nc.sync.dma_start(c1r[:], dr["c1r"][:, :])
        c2t = persist.tile([1, 1], dt.float32, tag="c2", name="c2")
        nc.sync.dma_start(c2t[:], dr["c2"][:, :])
        if not inj_none:
            b1r = persist.tile([1, 1024], dt.bfloat16, tag="b1r", name="b1r")
            nc.sync.dma_start(b1r[:], dr["b1r"][:, :])
            ind32 = persist.tile([128, 2048], dt.bfloat16, tag="ind32", name="ind32")
            nc.sync.dma_start(ind32[:], dr["ind32"][:, :])
            onesb = persist.tile([1, 128], dt.bfloat16, tag="onesb", name="onesb")
            nc.vector.memset(onesb[:], 1.0)
        if not inj_all:
            b1s = [persist.tile([128, 1], dt.float32, tag=f"b1_{s}", name=f"b1_{s}")
                   for s in range(8)]
            for s in range(8):
                nc.sync.dma_start(b1s[s][:], dr["b1"][128 * s:128 * (s + 1), :])

        # broadcast constants across partitions (gpsimd)
        c1_128 = persist.tile([128, 2048], dt.bfloat16, tag="c1_128", name="c1_128")
        nc.gpsimd.partition_broadcast(c1_128[:], c1r[:])
        c2_128 = persist.tile([128, 1], dt.float32, tag="c2_128", name="c2_128")
        nc.gpsimd.partition_broadcast(c2_128[:], c2t[:])

        # ---- persistent activations ----
        ghT = persist.tile([128, 2048], dt.bfloat16, tag="ghT", name="ghT")
        xhT = persist.tile([128, 2048], dt.bfloat16, tag="xhT", name="xhT")
        if not inj_none:
            a1n = [persist.tile([128, 1024], dt.bfloat16, tag=f"a1n_{g}", name=f"a1n_{g}")
                   for g in range(4)]
        if not inj_all:
            a1T = [persist.tile([128, 512], dt.bfloat16, tag=f"a1T_{s}", name=f"a1T_{s}")
                   for s in range(8)]
        a2n = [[persist.tile([64, 2048], dt.bfloat16, tag=f"a2n_{g}_{j}",
                             name=f"a2n_{g}_{j}") for j in range(2)]
               for g in range(4)]
        logT = persist.tile([128, 64], dt.float32, tag="logT", name="logT")

        # ---- working pools ----
        xq = ctx.enter_context(tc.tile_pool(name="xq", bufs=2))
        xb = ctx.enter_context(tc.tile_pool(name="xb", bufs=4 if BATCH_SOFTMAX else 3))
        gp = ctx.enter_context(tc.tile_pool(name="gp", bufs=3))
        h8p = ctx.enter_context(tc.tile_pool(name="h8p", bufs=2))
        up = ctx.enter_context(tc.tile_pool(name="up", bufs=4 if BATCH_SOFTMAX else 3))
        xup = ctx.enter_context(tc.tile_pool(name="xup", bufs=2))
        st = ctx.enter_context(tc.tile_pool(name="st", bufs=2))
        ps_c = ctx.enter_context(tc.tile_pool(name="ps_c", bufs=3, space="PSUM"))
        ps_f = ctx.enter_context(tc.tile_pool(name="ps_f", bufs=2, space="PSUM"))
        ps_e = ctx.enter_context(tc.tile_pool(name="ps_e", bufs=1, space="PSUM"))

        ghT_v = ghT[:].rearrange("p (d b) -> p d b", d=4)
        xhT_v = xhT[:].rearrange("p (d b) -> p d b", d=4)

        gts_by_g = {}
        quad = []           # [(rt, xbt)] awaiting batched softmax
        equad = {}          # quad id -> ep4 psum tile

        def emit_BA_load(g):
            gts = []
            for i in range(4):
                rt = 4 * g + i
                gt = gp.tile([128, 2048], dt.bfloat16, tag=f"gt_{i}",
                             name=f"gt_{i}")
                gts.append(gt)
                nc.sync.dma_start(gt[:], dr["gtb"][:, 2048 * rt:2048 * (rt + 1)])
            gts_by_g[g] = gts

        def emit_BA_gsum(g):
            for i in range(4):
                rt = 4 * g + i
                gt = gts_by_g[g][i]
                with nc.allow_low_precision("gsum bf16 out"):
                    nc.vector.reduce_sum(
                        ghT_v[:, :, 32 * rt:32 * (rt + 1)],
                        gt[:].rearrange("p (d b k) -> p d b k", d=4, k=K),
                        axis=mybir.AxisListType.X)

        def emit_BA_mm(g):
            # A-stage: a1n [b, h] for inject slices; a1T [h, b] for DVE-add ones
            if not inj_none:
                for half in range(2):
                    ap_h = ps_c.tile([128, 512], dt.float32, tag="c", name="c")
                    for d_ in range(4):
                        nc.tensor.matmul(ap_h[:, :],
                                         ghT_v[:, d_, 128 * g:128 * (g + 1)],
                                         w1qb[d_][:, 512 * half:512 * (half + 1)],
                                         start=(d_ == 0), stop=False)
                    nc.tensor.matmul(ap_h[:, :], onesb[:],
                                     b1r[:, 512 * half:512 * (half + 1)],
                                     start=False, stop=True)
                    nc.scalar.activation(a1n[g][:, 512 * half:512 * (half + 1)],
                                         ap_h[:, :], AF.Copy)
            if not inj_all:
                for sp in range(4):
                    ap_ = ps_c.tile([128, 512], dt.float32, tag="c", name="c")
                    for half in range(2):
                        s = 2 * sp + half
                        sub = ap_[:, 256 * half:256 * half + 128]
                        for d_ in range(4):
                            nc.tensor.matmul(sub,
                                             w1qb[d_][:, 128 * s:128 * (s + 1)],
                                             ghT_v[:, d_, 128 * g:128 * (g + 1)],
                                             start=(d_ == 0), stop=(d_ == 3))
                        nc.scalar.activation(a1T[s][:, 128 * g:128 * (g + 1)],
                                             sub, AF.Copy)

        def emit_xu(rt, xbt, u128):
            xu = xup.tile([128, 2048], dt.bfloat16, tag="xu", name="xu")
            # split the alpha-weighting: d-chunks 0,1 on gpsimd, 2,3 on DVE
            nc.gpsimd.tensor_tensor(
                xu[:, 0:1024].rearrange("p (d n) -> p d n", d=2),
                xbt[:, 0:1024].rearrange("p (d n) -> p d n", d=2),
                u128[:].unsqueeze(1).broadcast_to((128, 2, 512)), ALU.mult)
            nc.vector.tensor_tensor(
                xu[:, 1024:2048].rearrange("p (d n) -> p d n", d=2),
                xbt[:, 1024:2048].rearrange("p (d n) -> p d n", d=2),
                u128[:].unsqueeze(1).broadcast_to((128, 2, 512)), ALU.mult)
            if DEBUG_XHT and rt == 0:
                nc.sync.dma_start(dr["u_out"][:, :], u128[:])
                nc.sync.dma_start(dr["xu_out"][:, :], xu[:])
            v = xu[:].rearrange("p (d b t) -> p d b t", d=4, t=T)
            nc.vector.tensor_tensor(v[:, :, :, 0:16], v[:, :, :, 0:16],
                                    v[:, :, :, 16:32], ALU.add)
            nc.vector.tensor_tensor(v[:, :, :, 0:8], v[:, :, :, 0:8],
                                    v[:, :, :, 8:16], ALU.add)
            nc.vector.tensor_tensor(v[:, :, :, 0:4], v[:, :, :, 0:4],
                                    v[:, :, :, 4:8], ALU.add)
            nc.vector.tensor_tensor(v[:, :, :, 0:2], v[:, :, :, 0:2],
                                    v[:, :, :, 2:4], ALU.add)
            nc.vector.tensor_tensor(xhT_v[:, :, 16 * rt:16 * (rt + 1)],
                                    v[:, :, :, 0], v[:, :, :, 1], ALU.add)

        def emit_softmax_pair():
            # pair holds 2 rts sharing ep2 at partitions {0,32}
            ep2 = equad.pop("ep")
            un2 = st.tile([64, 512], dt.bfloat16, tag="un2", name="un2")
            nc.scalar.activation(un2[:], ep2[:], AF.Exp)
            srow = st.tile([64, 16], dt.float32, tag="srow", name="srow")
            nc.vector.reduce_sum(srow[:],
                                 un2[:].rearrange("p (b t) -> p b t", t=T),
                                 axis=mybir.AxisListType.X)
            rs = st.tile([64, 16], dt.bfloat16, tag="rs", name="rs")
            with nc.allow_low_precision("alpha norm bf16"):
                nc.vector.reciprocal(rs[:], srow[:])
            unn = st.tile([64, 512], dt.bfloat16, tag="unn", name="unn")
            nc.vector.tensor_tensor(
                unn[:].rearrange("p (b t) -> p b t", t=T),
                un2[:].rearrange("p (b t) -> p b t", t=T),
                rs[:].unsqueeze(-1).broadcast_to((64, 16, T)), ALU.mult)
            for rr, (rt_r, xbt_r) in enumerate(quad):
                u128 = up.tile([128, 512], dt.bfloat16, tag="u128", name="u128")
                nc.gpsimd.partition_broadcast(u128[:], unn[32 * rr:32 * rr + 1, :])
                emit_xu(rt_r, xbt_r, u128)
            quad.clear()

        def emit_softmax_single(rt, xbt, ep):
            u_row = st.tile([1, 512], dt.bfloat16, tag="u_row", name="u_row")
            with nc.allow_low_precision("u bf16"):
                nc.scalar.activation(u_row[:], ep[:], AF.Exp)
            srow = st.tile([1, 16], dt.float32, tag="srow1", name="srow1")
            nc.vector.reduce_sum(srow[:],
                                 u_row[:].rearrange("p (b t) -> p b t", t=T),
                                 axis=mybir.AxisListType.X)
            rs = st.tile([1, 16], dt.bfloat16, tag="rs1", name="rs1")
            with nc.allow_low_precision("alpha norm bf16"):
                nc.vector.reciprocal(rs[:], srow[:])
            un = st.tile([1, 512], dt.bfloat16, tag="un", name="un")
            nc.vector.tensor_tensor(
                un[:].rearrange("p (b t) -> p b t", t=T),
                u_row[:].rearrange("p (b t) -> p b t", t=T),
                rs[:].unsqueeze(-1).broadcast_to((1, 16, T)), ALU.mult)
            u128 = up.tile([128, 512], dt.bfloat16, tag="u128", name="u128")
            nc.gpsimd.partition_broadcast(u128[:], un[:])
            emit_xu(rt, xbt, u128)

        def emit_C_rt(rt):
            g = rt // 8
            q2 = (rt % 8) // 2
            parity = rt % 2
            x8 = [xq.tile([128, 1024], dt.float8e4, tag=f"x8_{p}",
                          name=f"x8_{p}") for p in range(2)]
            for p in range(2):
                nc.scalar.dma_start(
                    x8[p][:], dr["xt8"][128 * p:128 * (p + 1),
                                        1024 * rt:1024 * (rt + 1)])
            xbt = xb.tile([128, 2048], dt.bfloat16, tag="xb", name="xb")
            nc.sync.dma_start(xbt[:], dr["xtb"][:, 2048 * rt:2048 * (rt + 1)])

            h8s = [h8p.tile([128, 1024], dt.float8e4, tag=f"h8_{sp}",
                            name=f"h8_{sp}") for sp in range(4)]
            for sp in range(4):
                for half in range(2):
                    s = 2 * sp + half
                    hp = ps_c.tile([128, 512], dt.float32, tag="c", name="c")
                    sub = hp[:, :]
                    inject = s in INJ_SLICES
                    if inject:
                        # base partition 96 is rejected by bass; for q2==3 use a
                        # base-64 [64,*] stationary with the "hi" indicator half
                        if q2 < 3:
                            nc.tensor.matmul(
                                sub,
                                a1n[g][32 * q2:32 * (q2 + 1), 128 * s:128 * (s + 1)],
                                ind32[32 * q2:32 * (q2 + 1),
                                      512 * parity:512 * (parity + 1)],
                                start=True, stop=False)
                        else:
                            nc.tensor.matmul(
                                sub,
                                a1n[g][64:128, 128 * s:128 * (s + 1)],
                                ind32[64:128,
                                      1024 + 512 * parity:1024 + 512 * (parity + 1)],
                                start=True, stop=False)
                    for p in range(2):
                        nc.tensor.matmul(
                            sub,
                            w1x8[p][:, 256 * s:256 * (s + 1)].rearrange(
                                "p (k m) -> p k m", k=2),
                            x8[p][:].rearrange("p (k n) -> p k n", k=2),
                            start=(p == 0 and not inject), stop=(p == 1),
                            perf_mode=mybir.MatmulPerfMode.DoubleRow)
                    dst = h8s[sp][:, 512 * half:512 * half + 512]
                    if not inject:
                        h3 = sub.rearrange("p (b t) -> p b t", t=T)
                        a1b = a1T[s][:, 16 * rt:16 * (rt + 1)].unsqueeze(-1) \
                            .broadcast_to((128, 16, T))
                        nc.vector.tensor_tensor(h3, h3, a1b, ALU.add)
                        if (rt + s) % 8 in C_EVAC_DVE:
                            nc.vector.tensor_scalar(dst, sub, b1s[s][:, 0:1],
                                                    0.0, ALU.add, op1=ALU.max)
                        else:
                            nc.scalar.activation(dst, sub, AF.Relu,
                                                 bias=b1s[s][:])
                    else:
                        if (rt + s) % 8 in C_EVAC_DVE:
                            nc.vector.tensor_scalar(dst, sub, 0.0, 0.0,
                                                    ALU.max, op1=ALU.add)
                        else:
                            nc.scalar.activation(dst, sub, AF.Relu)

            if BATCH_SOFTMAX:
                r = rt % 2
                if r == 0:
                    equad["ep"] = ps_e.tile([64, 512], dt.float32, tag="e",
                                            name="e")
                ep2 = equad["ep"]
                for sp in range(4):
                    # r-variants live in separate 32-aligned column blocks so
                    # the DoubleRow weight load stays 16B-aligned
                    nc.tensor.matmul(
                        ep2[32 * r:32 * r + 1, :],
                        w28[:, 64 * sp + 32 * r:64 * sp + 32 * r + 32].rearrange(
                            "p (k m) -> p k m", k=2)[:, :, 0:1],
                        h8s[sp][:].rearrange("p (k n) -> p k n", k=2),
                        start=(sp == 0), stop=(sp == 3),
                        perf_mode=mybir.MatmulPerfMode.DoubleRow)
                quad.append((rt, xbt))
                if r == 1:
                    emit_softmax_pair()
            else:
                ep = ps_e.tile([1, 512], dt.float32, tag="e", name="e")
                for sp in range(4):
                    nc.tensor.matmul(
                        ep[:], w28[:, 64 * sp:64 * sp + 32].rearrange(
                            "p (k m) -> p k m", k=2)[:, :, 0:1],
                        h8s[sp][:].rearrange("p (k n) -> p k n", k=2),
                        start=(sp == 0), stop=(sp == 3),
                        perf_mode=mybir.MatmulPerfMode.DoubleRow)
                if DEBUG_XHT and rt == 0:
                    nc.sync.dma_start(dr["h8_out"][:, :], h8s[0][:])
                    nc.sync.dma_start(dr["a1n_out"][:, :], a1n[0][:])
                emit_softmax_single(rt, xbt, ep)

        def emit_E(g):
            esubs = [ps_f.tile([128, 1024], dt.float32, tag="f", name="f")
                     for _ in range(2)]
            for d_ in range(4):
                stat = xhT_v[:, d_, 128 * g:128 * (g + 1)]
                for q in range(4):
                    nc.tensor.matmul(esubs[q // 2][:, 512 * (q % 2):512 * (q % 2) + 512],
                                     stat,
                                     m1xb[d_][:, 512 * q:512 * (q + 1)],
                                     start=(d_ == 0), stop=(d_ == 3))
            for j in range(2):
                for jq in range(2):
                    nc.vector.tensor_tensor(
                        a2n[g][j][:, 1024 * jq:1024 * (jq + 1)],
                        esubs[jq][64 * j:64 * (j + 1), :],
                        c1_128[64 * j:64 * (j + 1), 1024 * jq:1024 * (jq + 1)],
                        ALU.add)

        def emit_F_rt(rt):
            g, i = rt // 4, rt % 4
            k_ = i
            gt = gts_by_g[g][i]
            acc = st.tile([128, 4 * n_acc], dt.float32, tag="acc", name="acc")
            for c in range(4):
                pqs = [ps_f.tile([128, 1024], dt.float32, tag="f", name="f")
                       for _ in range(2)]
                subs = [pqs[q // 2][:, 512 * (q % 2):512 * (q % 2) + 512]
                        for q in range(4)]
                # d outer / q inner: the gt stationary chunk is reused by
                # 4 consecutive matmuls (weight reload is the HW cost)
                for d_ in range(4):
                    stat = gt[:, 512 * d_ + 128 * c:512 * d_ + 128 * (c + 1)]
                    for q in range(4):
                        nc.tensor.matmul(
                            subs[q], stat,
                            m1gb[d_][:, 512 * q:512 * (q + 1)],
                            start=(d_ == 0), stop=False)
                jh, kh = k_ // 2, k_ % 2
                istat = bindfh[jh][32 * kh:32 * (kh + 1), 128 * c:128 * (c + 1)]
                for q in range(4):
                    nc.tensor.matmul(
                        subs[q], istat,
                        a2n[g][jh][32 * kh:32 * (kh + 1),
                                   512 * q:512 * (q + 1)],
                        start=False, stop=True)
                # accum-op outputs are dead: write in place (PSUM) or to dums
                for (blk, a, b, col, kind) in f_ranges:
                    src = pqs[blk][:, a:b]
                    accc = acc[:, c * n_acc + col:c * n_acc + col + 1]
                    if DUM_SBUF:
                        dum = st.tile([128, 1024], dt.float8e4, tag="dum",
                                      name="dum")
                        dst = dum[:, 0:b - a]
                    else:
                        dst = src
                    if kind == "pos":
                        if (rt * 4 + c) % 4 in F_POS_DVE:
                            nc.vector.tensor_scalar(
                                dst, src, 0.0, 0.0,
                                ALU.max, op1=ALU.add, accum_out=accc)
                        else:
                            nc.scalar.activation(dst, src, AF.Relu,
                                                 accum_out=accc)
                    else:
                        if F_NEG_ACT:
                            nc.scalar.activation(dst, src, AF.Relu,
                                                 scale=-1.0, accum_out=accc)
                        else:
                            nc.vector.tensor_scalar(
                                dst, src, 0.0, 0.0,
                                ALU.min, op1=ALU.add, accum_out=accc)
            # strips: pos cols minus neg cols (neg cols hold +sum(relu(-x))
            # under F_NEG_ACT, else signed sums that add directly)
            if F_NEG_ACT:
                pos_cols = [col for (_, _, _, col, kind) in f_ranges
                            if kind == "pos"]
                neg_cols = [col for (_, _, _, col, kind) in f_ranges
                            if kind == "neg"]
                # cols were emitted interleaved; rebuild per-c views
                # pos cols are contiguous [0, npos), neg [npos, n_acc) only if
                # ranges ordered pos-first; reorder acc columns accordingly
                npos = len(pos_cols)
                tpos = st.tile([128, 4], dt.float32, tag="tpos", name="tpos")
                tneg = st.tile([128, 4], dt.float32, tag="tneg", name="tneg")
                nc.vector.reduce_sum(
                    tpos[:], acc[:].rearrange("p (c n) -> p c n", n=n_acc)[
                        :, :, 0:npos],
                    axis=mybir.AxisListType.X)
                nc.vector.reduce_sum(
                    tneg[:], acc[:].rearrange("p (c n) -> p c n", n=n_acc)[
                        :, :, npos:n_acc],
                    axis=mybir.AxisListType.X)
                nc.vector.tensor_tensor(logT[:, 4 * rt:4 * (rt + 1)],
                                        tpos[:], tneg[:], ALU.subtract)
            else:
                nc.vector.reduce_sum(
                    logT[:, 4 * rt:4 * (rt + 1)],
                    acc[:].rearrange("p (c n) -> p c n", n=n_acc),
                    axis=mybir.AxisListType.X)

        first = True
        for _rep in range(reps):
            emit_BA_load(0)
            emit_BA_gsum(0)
            emit_BA_mm(0)
            if first:
                for d_ in range(4):
                    nc.scalar.dma_start(m1xb[d_][:],
                                        dr["m1xb"][128 * d_:128 * (d_ + 1), :])
                for d_ in range(4):
                    nc.sync.dma_start(m1gb[d_][:],
                                      dr["m1gb"][128 * d_:128 * (d_ + 1), :])
                first = False
            for g in range(4):
                for i in range(8):
                    emit_C_rt(8 * g + i)
                    if i == 1 and g + 1 < 4:
                        emit_BA_load(g + 1)
                    if i == 4 and g + 1 < 4:
                        emit_BA_gsum(g + 1)
                    if g > 0 and i % 2 == 1:
                        emit_F_rt(4 * (g - 1) + i // 2)
                if g + 1 < 4:
                    emit_BA_mm(g + 1)
                emit_E(g)
            for i in range(4):
                emit_F_rt(12 + i)

            # logits = strips/S + c2
            nc.vector.tensor_scalar(logT[:], logT[:], 1.0 / S_F,
                                    c2_128[:, 0:1], ALU.mult, op1=ALU.add)
            nc.sync.dma_start(dr["out"][:, :], logT[:])
            if DEBUG_XHT:
                nc.sync.dma_start(dr["xht_out"][:, :], xhT[:])
                nc.sync.dma_start(dr["ght_out"][:, :], ghT[:])
                nc.sync.dma_start(dr["a2n_out"][:, :], a2n[0][0][:])


_NC_CACHE = {}


def _prep_shared(W1, b1, w2, M1, c1, m2, c2):
    W1q = (W1[:D] / K).astype(np.float32)
    W1x = W1[D:]
    pos = m2 >= 0
    sigma = np.concatenate([np.nonzero(pos)[0], np.nonzero(~pos)[0]])
    P = int(pos.sum())
    sm2 = m2 * S_F   # signed scaling, fp8-range boost; undone in final strip op

    w1x8 = np.ascontiguousarray(
        W1x.reshape(2, 2, 128, 8, 128).transpose(0, 2, 3, 1, 4)
        .reshape(256, 2048)).astype(F8)
    w1qb = np.ascontiguousarray(W1q).astype(BF)
    w28 = np.zeros((128, 256), np.float32)
    for sp in range(4):
        for r in range(2):
            w28[:, 64 * sp + 32 * r] = w2[128 * 2 * sp:128 * (2 * sp + 1)]
            w28[:, 64 * sp + 32 * r + 16] = w2[128 * (2 * sp + 1):128 * (2 * sp + 2)]
    w28 = w28.astype(F8)
    m1xb = np.ascontiguousarray((M1[:D][:, sigma] * sm2[sigma])).astype(BF)
    m1gb = np.ascontiguousarray((M1[D:][:, sigma] * sm2[sigma])).astype(BF)
    c1r = np.ascontiguousarray((c1[sigma] * sm2[sigma])[None, :]).astype(BF)
    b1h = np.ascontiguousarray(b1[:, None]).astype(np.float32)
    b1r = np.ascontiguousarray(b1[None, :]).astype(BF)

    # cols [0:1024]: ind32[p, 512*parity + row] = 1 iff p%32 == 16*parity + row//32
    #   (used with 32-row stationaries at base 0/32/64)
    # cols [1024:2048]: hi variant for the base-64 [64,*] stationary (q2==3):
    #   nonzero only for p%64 >= 32, selecting p%64-32 == 16*parity + row//32
    ind32 = np.zeros((128, 2048), np.float32)
    for p in range(128):
        for parity in range(2):
            b = p % 32 - 16 * parity
            if 0 <= b < 16:
                ind32[p, 512 * parity + 32 * b:512 * parity + 32 * (b + 1)] = 1.0
            if p % 64 >= 32:
                b = p % 64 - 32 - 16 * parity
                if 0 <= b < 16:
                    ind32[p, 1024 + 512 * parity + 32 * b:
                          1024 + 512 * parity + 32 * (b + 1)] = 1.0
    ind32 = ind32.astype(BF)

    bindf = np.zeros((128, 512), np.float32)
    for row in range(128):
        b = row % 32
        bindf[row, 16 * b:16 * (b + 1)] = 1.0
    bindf = bindf.astype(BF)
    c2h = np.asarray(c2, np.float32).reshape(1, 1)
    shared = dict(W1X8=w1x8, W1QB=w1qb, W28=w28, M1XB=m1xb, M1GB=m1gb,
                  C1R=c1r, B1=b1h, B1R=b1r, IND32=ind32, BINDF=bindf, C2=c2h)
    return shared, P


def _prep_core(Xc, Gc):
    # Xc [NX, D] fp32; Gc [NG, D] fp32
    xt8 = np.ascontiguousarray(
        Xc.reshape(RT_X, 512, 2, 2, 128).transpose(2, 4, 0, 3, 1)
        .reshape(256, 2 * NX)).astype(F8)
    xtb = np.ascontiguousarray(
        Xc.reshape(RT_X, 512, 4, 128).transpose(3, 0, 2, 1)
        .reshape(128, 4 * NX)).astype(BF)
    gtb = np.ascontiguousarray(
        Gc.reshape(RT_G, 512, 4, 128).transpose(3, 0, 2, 1)
        .reshape(128, 4 * NG)).astype(BF)
    return dict(XT8=xt8, XTB=xtb, GTB=gtb)


def kernel(**inputs):
    X = np.asarray(inputs["X"], dtype=np.float32)
    G = np.asarray(inputs["G"], dtype=np.float32)
    W1 = np.asarray(inputs["W1"], dtype=np.float32)
    b1 = np.asarray(inputs["b1"], dtype=np.float32)
    w2 = np.asarray(inputs["w2"], dtype=np.float32)
    M1 = np.asarray(inputs["M1"], dtype=np.float32)
    c1 = np.asarray(inputs["c1"], dtype=np.float32)
    m2 = np.asarray(inputs["m2"], dtype=np.float32)
    c2 = np.asarray(inputs["c2"], dtype=np.float32)

    shared, P = _prep_shared(W1, b1, w2, M1, c1, m2, c2)
    if _NC_CACHE.get("P") != P:
        _NC_CACHE["nc"] = build_nc(P)
        _NC_CACHE["P"] = P
    nc = _NC_CACHE["nc"]

    in_maps = []
    for c in range(NCORES):
        m = dict(shared)
        m.update(_prep_core(
            X[c * B_CORE:(c + 1) * B_CORE].reshape(NX, D),
            G[c * B_CORE:(c + 1) * B_CORE].reshape(NG, D)))
        in_maps.append(m)

    _NC_CACHE["in_maps"] = in_maps
    res = run_bass_kernel_spmd(nc, in_maps, list(range(NCORES)))
    outs = []
    for c in range(NCORES):
        r = np.asarray(res.results[c]["out"], np.float32)   # [128, 64]
        outs.append(r.T.reshape(B_CORE, K))                 # rt-major -> rows
    return np.concatenate(outs, axis=0)
